# revision 30
# baseline (speedup 1.0000x reference)
"""CAN per-sample 2-layer MLP kernel for Trainium2 (8 NeuronCores, SPMD).

Computation (per sample b):
    x = user_emb[b]                           # (50, 16)
    W0, b0, W1, b1 unpacked from item_emb[b]  # (16,16),(16,),(16,16),(16,)
    y = relu(relu(x @ W0 + b0) @ W1 + b1)     # (50, 16)

Mapping:
  * Pure data parallel over 8 cores (2048 samples each).
  * Host packs x^T per sample with an appended ones-row (homogeneous
    coordinates); bias is folded into a 17x17 Wt0 = [[W0,0],[b0,1]] and a
    17x16 Wt1 = [[W1],[b1]] so `x_t @ Wt` applies bias, and the ones row
    self-propagates through layer 1 (relu(1)=1).
  * On chip: 4 samples share a 128-partition tile at 32-row strides so each
    sample's K=17 matmul runs at its own PE tile_position (32j, 32j); the
    four matmuls execute concurrently in distinct array quadrants.
  * x and W are interleaved into ONE combined DRAM tensor, loaded with a
    single 4-dim-AP DMA per batch (and one 4-dim-AP store per batch).
  * Sync-wait budget (walrus codegen): a DMACopy fits only 1 wait and it is
    always consumed by the HWDGE lane-reuse wait. Data/slot-release waits
    that would otherwise land on DMAs are absorbed by nop "gate"
    instructions on the SP queue (sync dep on the producer); Tile's
    per-proc-minimal pass then drops them from the DMA itself. relu1/relu2
    alternate ACT/DVE per batch with a parity that makes every psum-slot
    release wait coincide with an existing same-engine data wait.
"""

from contextlib import ExitStack

import numpy as np

import concourse.bass as bass
import concourse.mybir as mybir
from concourse import tile
from concourse.bass_utils import run_bass_kernel_spmd
from concourse.tile_rust import add_dep_helper

# Problem constants (hardcoded per contract)
B, N, D = 16384, 50, 16
NCORES = 8
BC = B // NCORES            # 2048 samples per core
K = D + 1                   # 17 rows: 16 features + homogeneous ones row
WC = K + D                  # 33 weight cols: 17 (layer0 incl ones col) + 16 (layer1)
CC = N + WC                 # 83 combined cols per quad-slot: x (50) then w (33)
QUADS = BC // 4             # 512 quads of 4 samples per core

F32 = mybir.dt.float32


def _strip_covered_waits(nc):
    """Remove, from DMACopy instructions only, semaphore waits already
    guaranteed by an earlier instruction on the same engine queue (the gate
    NoOps). Walrus codegen fits only 1 sync wait on a DMACopy, and Tile's
    slot-release waits bypass its own per-proc-minimal clearing. Only sems
    that are never decremented/reset anywhere are considered (barrier sems
    go down; a monotonic coverage argument would be unsound for them)."""
    for fn in nc.m.functions:
        for blk in fn.blocks:
            seen = {}  # (engine, sem_id) -> max waited value
            for ins in blk.instructions:
                si = ins.sync_info
                if si is None:
                    continue
                eng = ins.engine
                strippable = type(ins).__name__ == "InstDMACopy"
                kept = []
                changed = False
                for w in si.on_wait:
                    if (
                        strippable
                        and w.wait_mode == "sem-ge-imm"
                        and w.wait_reg is None
                        and seen.get((eng, w.id), -1) >= w.wait_value
                    ):
                        changed = True
                        continue
                    kept.append(w)
                for w in kept:
                    if w.wait_mode == "sem-ge-imm" and w.wait_reg is None:
                        key = (eng, w.id)
                        if seen.get(key, -1) < w.wait_value:
                            seen[key] = w.wait_value
                # A non-increment update (barrier resets/decrements) kills
                # coverage for that sem on every engine from this point on.
                for u in si.on_update:
                    if u.update_mode != "sem-add-imm" or (
                        u.update_value is not None and u.update_value < 0
                    ):
                        for key in [k for k in seen if k[1] == u.id]:
                            del seen[key]
                if changed:
                    ins.sync_info = mybir.SyncInfo(
                        on_wait=kept, on_update=si.on_update
                    )


_WS_COUNT = [0]


def _split_excess_waits(nc, cap=1):
    """Walrus codegen fits very few inline sync waits per instruction (a
    DMACopy or self-loading f32 Matmult: 1). Move excess waits onto NoOp
    instructions inserted immediately before, on the same engine queue -
    semantically identical (the sequencer executes them first)."""
    for fn in nc.m.functions:
        for blk in fn.blocks:
            insts = blk.instructions
            i = 0
            while i < len(insts):
                ins = insts[i]
                si = ins.sync_info
                if si is None or len(si.on_wait) <= cap:
                    i += 1
                    continue
                waits = list(si.on_wait)
                keep, extra = waits[-cap:], waits[:-cap]
                ins.sync_info = mybir.SyncInfo(on_wait=keep, on_update=si.on_update)
                for w in extra:
                    _WS_COUNT[0] += 1
                    nop = mybir.InstNoOp(name=f"I-ws{_WS_COUNT[0]}", ins=[], outs=[])
                    nop.engine = ins.engine
                    nop.sync_info = mybir.SyncInfo(on_wait=[w], on_update=[])
                    insts.insert(i, nop)
                    i += 1
                i += 1


def build_nc(nq=QUADS, g=8, dt=F32, sim_mode=False, fused_in=None, fused_out=None, cbufs=3, obufs=3):
    """Build the per-core Bass program.

    DRAM tensors (per core):
      ch [4*K, nq, CC] : ch[K*j+d, q, 0:50]  = x^T row d of sample (4q+j)
                         ch[K*j+d, q, 50:83] = [Wt0[d] | Wt1[d]]
      yh [4*D, nq, N]  : yh[D*j+e, q, n] = y[4q+j][n, e]   (output)

    sim_mode: CoreSim's shadow tracker cannot handle multi-partition-stride
    DMA APs (false races) nor never-written psum gap rows (false uninit).
    sim_mode splits DMAs per 32-row group and memsets psum tiles; compute
    is identical. HW builds use the fused 4-dim-AP DMAs (HW-verified).
    """
    assert nq % g == 0
    nbatch = nq // g
    cf = CC * g              # combined data cols per batch
    xf = N * g               # psum/ht/yt data cols per batch
    # Pad SBUF row widths so the AP optimizer cannot merge the per-group
    # partition dim into one flat free run (the HW DGE does not roll a
    # free-dim run across partition boundaries).
    cfp = cf + 8
    xfp = xf + 8

    nc = bass.Bass(
        "TRN2",
        target_bir_lowering=False,
        debug=False,
        # The wait-splitting post-pass inserts NoOps the race detector's
        # fake-sem bookkeeping doesn't know about.
        detect_race_conditions=False,
    )
    # Batch-major host layout: each batch's block is contiguous in exactly
    # the SBUF tile order, so every DMA has a contiguous DRAM side.
    ch = nc.dram_tensor("ch", [nbatch, 4 * K, cf], dt, kind="ExternalInput")
    yh = nc.dram_tensor("yh", [nbatch, 4 * D, xf], F32, kind="ExternalOutput")

    relu = mybir.ActivationFunctionType.Relu
    CBUFS = cbufs

    with tile.TileContext(nc) as tc, ExitStack() as ctx:
        cpool = ctx.enter_context(tc.tile_pool(name="cpool", bufs=CBUFS))
        hpool = ctx.enter_context(tc.tile_pool(name="hpool", bufs=3))
        ypool = ctx.enter_context(tc.tile_pool(name="ypool", bufs=obufs))
        pspool = ctx.enter_context(tc.tile_pool(name="ps", bufs=3, space="PSUM"))

        last_l2mm = {}       # batch -> last layer-2 matmul instruction
        relu2s = {}          # batch -> relu2 instruction

        for bi in range(nbatch):
            q0 = bi * g

            # --- input DMA, gated so it carries only its lane wait ---
            if bi >= CBUFS and last_l2mm.get(bi - CBUFS) is not None:
                gate_in = nc.sync.nop(hint=f"gate_in_{bi}")
                add_dep_helper(gate_in.ins, last_l2mm[bi - CBUFS].ins, sync=True,
                               reason="absorb ct slot-release wait")
            else:
                gate_in = None

            ct = cpool.tile([128, cfp], dt, name="ct")
            cbase = bi * 4 * K * cf
            if True:  # SBUF-side DMA APs must be 2D (DIRECT2D struct)
                in_dmas = []
                for j in range(4):
                    in_dmas.append(nc.sync.dma_start(
                        bass.AP(ct.tensor, 32 * j * cfp, [[cfp, K], [1, cf]]),
                        bass.AP(ch, cbase + j * K * cf, [[cf, K], [1, cf]]),
                    ))
                first_in = in_dmas[0]
            else:
                first_in = nc.sync.dma_start(
                    bass.AP(ct.tensor, 0, [[32 * cfp, 4], [cfp, K], [1, cf]]),
                    bass.AP(ch, cbase, [[K * cf, 4], [cf, K], [1, cf]]),
                )
            if gate_in is not None:
                add_dep_helper(first_in.ins, gate_in.ins, sync=False,
                               reason="SP order: in-DMA after gate")

            # --- layer 1 matmuls ---
            ps1 = pspool.tile([128, xf], F32, name="ps1")
            if sim_mode:
                nc.vector.memset(ps1[:, :], 0.0)
            for q in range(g):
                for j in range(4):
                    nc.tensor.matmul(
                        bass.AP(ps1.tensor, 32 * j * xf + q * N, [[xf, K], [1, N]]),
                        bass.AP(ct.tensor, 32 * j * cfp + q * CC + N, [[cfp, K], [1, K]]),
                        bass.AP(ct.tensor, 32 * j * cfp + q * CC, [[cfp, K], [1, N]]),
                        start=True,
                        stop=True,
                        tile_position=(32 * j, 32 * j),
                    )

            # --- relu layer 1 (alternating engine; ones row stays 1) ---
            ht = hpool.tile([128, xf], dt, name="ht")
            if bi % 2 == 0:
                nc.scalar.activation(ht[:, :], ps1[:, :], relu)
            else:
                nc.vector.tensor_scalar_max(ht[:, :], ps1[:, :], 0.0)

            # --- layer 2 matmuls ---
            ps2 = pspool.tile([128, xf], F32, name="ps2")
            if sim_mode:
                nc.vector.memset(ps2[:, :], 0.0)
            mm = None
            for q in range(g):
                for j in range(4):
                    mm = nc.tensor.matmul(
                        bass.AP(ps2.tensor, 32 * j * xf + q * N, [[xf, D], [1, N]]),
                        bass.AP(ct.tensor, 32 * j * cfp + q * CC + N + K, [[cfp, K], [1, D]]),
                        bass.AP(ht.tensor, 32 * j * xf + q * N, [[xf, K], [1, N]]),
                        start=True,
                        stop=True,
                        tile_position=(32 * j, 32 * j),
                    )
            last_l2mm[bi] = mm

            # --- relu layer 2 (opposite engine of relu1 this batch) ---
            yt = ypool.tile([128, xfp], F32, name="yt")
            if bi % 2 == 0:
                r2 = nc.vector.tensor_scalar_max(yt[:, :xf], ps2[:, :], 0.0)
            else:
                r2 = nc.scalar.activation(yt[:, :xf], ps2[:, :], relu)
            relu2s[bi] = r2

            # --- output DMA, gated on relu2 completion ---
            gate_out = nc.sync.nop(hint=f"gate_out_{bi}")
            add_dep_helper(gate_out.ins, r2.ins, sync=True,
                           reason="absorb relu2 data wait")
            ybase = bi * 4 * D * xf
            if True:  # 2D SBUF APs only
                for j in range(4):
                    od = nc.sync.dma_start(
                        bass.AP(yh, ybase + j * D * xf, [[xf, D], [1, xf]]),
                        bass.AP(yt.tensor, 32 * j * xfp, [[xfp, D], [1, xf]]),
                    )
                    if j == 0:
                        add_dep_helper(od.ins, gate_out.ins, sync=False,
                                       reason="SP order: out-DMA after gate")
            else:
                od = nc.sync.dma_start(
                    bass.AP(yh, ybase, [[D * xf, 4], [xf, D], [1, xf]]),
                    bass.AP(yt.tensor, 0, [[32 * xfp, 4], [xfp, D], [1, xf]]),
                )
                add_dep_helper(od.ins, gate_out.ins, sync=False,
                               reason="SP order: out-DMA after gate")

    _strip_covered_waits(nc)
    _split_excess_waits(nc)
    return nc


def pack_inputs(user_emb, item_emb, nq=QUADS, g=8, dt=np.float32):
    """Shard + lay out inputs for the 8 cores. Returns list of in_maps.

    Layout per core: ch[batch, 4*K, g*CC] where batch = bi, partition-row
    (K*j+d), free (qq*CC + c) holds sample (bi*g+qq)*4+j's row d, col c.
    """
    ncores = NCORES
    nbatch = nq // g
    x = np.ascontiguousarray(user_emb, dtype=np.float32)
    ie = np.ascontiguousarray(item_emb, dtype=np.float32)
    btot = ncores * nq * 4

    comb = np.empty((btot, K, CC), dtype=np.float32)
    # x^T with ones row
    comb[:, :D, :N] = x[:btot].transpose(0, 2, 1)
    comb[:, D, :N] = 1.0
    # Wt0 | Wt1
    w = comb[:, :, N:]
    w[:, :D, :D] = ie[:btot, : D * D].reshape(btot, D, D)          # W0
    w[:, D, :D] = ie[:btot, D * D : D * D + D]                     # b0
    w[:, :D, D] = 0.0
    w[:, D, D] = 1.0                                               # ones col
    off = D * (D + 1)
    w[:, :D, K : K + D] = ie[:btot, off : off + D * D].reshape(btot, D, D)  # W1
    w[:, D, K : K + D] = ie[:btot, off + D * D : off + D * D + D]  # b1

    chs = (
        comb.reshape(ncores, nbatch, g, 4, K, CC)
        .transpose(0, 1, 3, 4, 2, 5)       # c, bi, j, d, qq, col
        .astype(dt, copy=False)
    )
    return [
        {"ch": np.ascontiguousarray(chs[c]).reshape(nbatch, 4 * K, g * CC)}
        for c in range(ncores)
    ]


def unpack_output(results, nq=QUADS, g=8):
    """results: per-core {"yh": [nbatch, 4*D, g*N]} -> full (B, N, D) f32."""
    nbatch = nq // g
    yh = np.stack([r["yh"] for r in results])   # [8, nbatch, 64, g*50]
    y = (
        yh.reshape(NCORES, nbatch, 4, D, g, N)
        .transpose(0, 1, 4, 2, 5, 3)            # c, bi, qq, j, n, e
    )
    return np.ascontiguousarray(y.reshape(NCORES * nq * 4, N, D))


_NC_CACHE = {}


def _get_nc(key=(QUADS, 8)):
    if key not in _NC_CACHE:
        nq, g = key
        _NC_CACHE[key] = build_nc(nq=nq, g=g)
    return _NC_CACHE[key]


def kernel(user_emb, item_emb):
    nc = _get_nc()
    in_maps = pack_inputs(user_emb, item_emb)
    res = run_bass_kernel_spmd(nc, in_maps, core_ids=list(range(NCORES)))
    return unpack_output(res.results)


# revision 31
# speedup vs baseline: 1.0224x; 1.0224x over previous
"""CAN per-sample 2-layer MLP kernel for Trainium2 (8 NeuronCores, SPMD).

Computation (per sample b):
    x = user_emb[b]                           # (50, 16)
    W0, b0, W1, b1 unpacked from item_emb[b]  # (16,16),(16,),(16,16),(16,)
    y = relu(relu(x @ W0 + b0) @ W1 + b1)     # (50, 16)

Mapping:
  * Pure data parallel over 8 cores (2048 samples each).
  * Host packs x^T per sample with an appended ones-row (homogeneous
    coordinates); bias is folded into a 17x17 Wt0 = [[W0,0],[b0,1]] and a
    17x16 Wt1 = [[W1],[b1]] so `x_t @ Wt` applies bias, and the ones row
    self-propagates through layer 1 (relu(1)=1).
  * On chip: 4 samples share a 128-partition tile at 32-row strides so each
    sample's K=17 matmul runs at its own PE tile_position (32j, 32j); the
    four matmuls execute concurrently in distinct array quadrants.
  * x and W interleave in ONE combined DRAM tensor, batch-major so every
    DMA has a contiguous DRAM side. DMA batches are large (G quads) to
    amortize the ~2us per-lane completion latency; PSUM works in GS-quad
    sub-batches (one bank per tile). Input DMAs are issued 2 batches ahead
    on an explicitly chained SP queue so loads overlap compute.
  * Walrus codegen caps inline sync waits (DMACopy/Matmult: 1): a post-pass
    moves excess waits onto NoOps inserted before the instruction on the
    same queue. SBUF-side DMA APs must stay 2D ([row, nparts], [1, run]) -
    the DIRECT2D DMA struct cannot roll a free run across partitions.
"""

from contextlib import ExitStack

import numpy as np

import concourse.bass as bass
import concourse.mybir as mybir
from concourse import tile
from concourse.bass_utils import run_bass_kernel_spmd
from concourse.tile_rust import add_dep_helper

# Problem constants (hardcoded per contract)
B, N, D = 16384, 50, 16
NCORES = 8
BC = B // NCORES            # 2048 samples per core
K = D + 1                   # 17 rows: 16 features + homogeneous ones row
WC = K + D                  # 33 weight cols: 17 (layer0 incl ones col) + 16 (layer1)
CC = N + WC                 # 83 combined cols per quad-slot: x (50) then w (33)
QUADS = BC // 4             # 512 quads of 4 samples per core
G = 32                      # quads per DMA batch
GS = 8                      # quads per PSUM sub-batch (one bank)

F32 = mybir.dt.float32


def _strip_covered_waits(nc):
    """Remove, from DMACopy instructions, semaphore waits already guaranteed
    by an earlier instruction on the same engine queue. Coverage is killed
    for a sem from the point of any non-increment update (barrier resets)."""
    for fn in nc.m.functions:
        for blk in fn.blocks:
            seen = {}
            for ins in blk.instructions:
                si = ins.sync_info
                if si is None:
                    continue
                eng = ins.engine
                strippable = type(ins).__name__ == "InstDMACopy"
                kept = []
                changed = False
                for w in si.on_wait:
                    if (
                        strippable
                        and w.wait_mode == "sem-ge-imm"
                        and w.wait_reg is None
                        and seen.get((eng, w.id), -1) >= w.wait_value
                    ):
                        changed = True
                        continue
                    kept.append(w)
                for w in kept:
                    if w.wait_mode == "sem-ge-imm" and w.wait_reg is None:
                        key = (eng, w.id)
                        if seen.get(key, -1) < w.wait_value:
                            seen[key] = w.wait_value
                for u in si.on_update:
                    if u.update_mode != "sem-add-imm" or (
                        u.update_value is not None and u.update_value < 0
                    ):
                        for key in [k for k in seen if k[1] == u.id]:
                            del seen[key]
                if changed:
                    ins.sync_info = mybir.SyncInfo(
                        on_wait=kept, on_update=si.on_update
                    )


_WS_COUNT = [0]


def _split_excess_waits(nc, cap=1):
    """Move excess inline waits onto NoOps inserted immediately before, on
    the same engine queue - semantically identical (sequencers execute
    waits in order)."""
    for fn in nc.m.functions:
        for blk in fn.blocks:
            insts = blk.instructions
            i = 0
            while i < len(insts):
                ins = insts[i]
                si = ins.sync_info
                if si is None or len(si.on_wait) <= cap:
                    i += 1
                    continue
                waits = list(si.on_wait)
                keep, extra = waits[-cap:], waits[:-cap]
                ins.sync_info = mybir.SyncInfo(on_wait=keep, on_update=si.on_update)
                for w in extra:
                    _WS_COUNT[0] += 1
                    nop = mybir.InstNoOp(name=f"I-ws{_WS_COUNT[0]}", ins=[], outs=[])
                    nop.engine = ins.engine
                    nop.sync_info = mybir.SyncInfo(on_wait=[w], on_update=[])
                    insts.insert(i, nop)
                    i += 1
                i += 1


def build_nc(nq=QUADS, g=G, gs=GS, dt=F32, sim_mode=False):
    """Build the per-core Bass program.

    DRAM (per core), batch-major:
      ch [nbatch, 4*K, g*CC] : row K*j+d, col qq*CC+c = sample (bi*g+qq)*4+j
      yh [nbatch, 4*D, g*N]  : row D*j+e, col qq*N+n  = y[n, e] of same
    """
    assert nq % g == 0 and g % gs == 0
    nbatch = nq // g
    nsub = g // gs
    cf = CC * g              # ct data cols per batch
    xf = N * g               # yt data cols per batch
    sf = N * gs              # psum/ht cols per sub-batch
    cfp = cf + 8             # padded row widths: keep SBUF DMA APs 2D
    xfp = xf + 8

    nc = bass.Bass(
        "TRN2",
        target_bir_lowering=False,
        debug=False,
        detect_race_conditions=False,  # post-pass NoOps confuse its bookkeeping
    )
    ch = nc.dram_tensor("ch", [nbatch, 4 * K, cf], dt, kind="ExternalInput")
    yh = nc.dram_tensor("yh", [nbatch, 4 * D, xf], F32, kind="ExternalOutput")

    relu = mybir.ActivationFunctionType.Relu

    with tile.TileContext(nc) as tc, ExitStack() as ctx:
        cpool = ctx.enter_context(tc.tile_pool(name="cpool", bufs=3))
        hpool = ctx.enter_context(tc.tile_pool(name="hpool", bufs=3))
        ypool = ctx.enter_context(tc.tile_pool(name="ypool", bufs=2))
        pspool = ctx.enter_context(tc.tile_pool(name="ps", bufs=4, space="PSUM"))

        prev_sp = [None]

        def sp_chain(inst):
            # Pin SP issue order to emission order so prefetched loads are
            # dispatched before later batches' stores.
            if prev_sp[0] is not None:
                add_dep_helper(inst.ins, prev_sp[0].ins, sync=False,
                               reason="SP issue order")
            prev_sp[0] = inst
            return inst

        cts = {}

        def emit_in_dma(bi):
            ct = cpool.tile([128, cfp], dt, name="ct")
            cts[bi] = ct
            for j in range(4):
                sp_chain(nc.sync.dma_start(
                    bass.AP(ct.tensor, 32 * j * cfp, [[cfp, K], [1, cf]]),
                    bass.AP(ch, (bi * 4 * K + j * K) * cf, [[cf, K], [1, cf]]),
                ))

        # 2-deep prefetch prologue
        emit_in_dma(0)
        if nbatch > 1:
            emit_in_dma(1)

        for bi in range(nbatch):
            if bi + 2 < nbatch:
                emit_in_dma(bi + 2)
            ct = cts.pop(bi)

            yt = ypool.tile([128, xfp], F32, name="yt")
            for s in range(nsub):
                ps1 = pspool.tile([128, sf], F32, name="ps1")
                if sim_mode:
                    nc.vector.memset(ps1[:, :], 0.0)
                for q in range(gs):
                    qq = s * gs + q
                    for j in range(4):
                        nc.tensor.matmul(
                            bass.AP(ps1.tensor, 32 * j * sf + q * N, [[sf, K], [1, N]]),
                            bass.AP(ct.tensor, 32 * j * cfp + qq * CC + N, [[cfp, K], [1, K]]),
                            bass.AP(ct.tensor, 32 * j * cfp + qq * CC, [[cfp, K], [1, N]]),
                            start=True,
                            stop=True,
                            tile_position=(32 * j, 32 * j),
                        )

                ht = hpool.tile([128, sf], dt, name="ht")
                nc.scalar.activation(ht[:, :], ps1[:, :], relu)

                ps2 = pspool.tile([128, sf], F32, name="ps2")
                if sim_mode:
                    nc.vector.memset(ps2[:, :], 0.0)
                for q in range(gs):
                    qq = s * gs + q
                    for j in range(4):
                        nc.tensor.matmul(
                            bass.AP(ps2.tensor, 32 * j * sf + q * N, [[sf, D], [1, N]]),
                            bass.AP(ct.tensor, 32 * j * cfp + qq * CC + N + K, [[cfp, K], [1, D]]),
                            bass.AP(ht.tensor, 32 * j * sf + q * N, [[sf, K], [1, N]]),
                            start=True,
                            stop=True,
                            tile_position=(32 * j, 32 * j),
                        )

                # relu2 writes this sub-batch's slice of yt (DVE)
                nc.vector.tensor_scalar_max(
                    bass.AP(yt.tensor, s * sf, [[xfp, 128], [1, sf]]),
                    ps2[:, :],
                    0.0,
                )

            for j in range(4):
                sp_chain(nc.sync.dma_start(
                    bass.AP(yh, (bi * 4 * D + j * D) * xf, [[xf, D], [1, xf]]),
                    bass.AP(yt.tensor, 32 * j * xfp, [[xfp, D], [1, xf]]),
                ))

    _strip_covered_waits(nc)
    _split_excess_waits(nc)
    return nc


def pack_inputs(user_emb, item_emb, nq=QUADS, g=G, dt=np.float32):
    """Shard + lay out inputs for the 8 cores. Returns list of in_maps."""
    ncores = NCORES
    nbatch = nq // g
    x = np.ascontiguousarray(user_emb, dtype=np.float32)
    ie = np.ascontiguousarray(item_emb, dtype=np.float32)
    btot = ncores * nq * 4

    comb = np.empty((btot, K, CC), dtype=np.float32)
    comb[:, :D, :N] = x[:btot].transpose(0, 2, 1)
    comb[:, D, :N] = 1.0
    w = comb[:, :, N:]
    w[:, :D, :D] = ie[:btot, : D * D].reshape(btot, D, D)          # W0
    w[:, D, :D] = ie[:btot, D * D : D * D + D]                     # b0
    w[:, :D, D] = 0.0
    w[:, D, D] = 1.0                                               # ones col
    off = D * (D + 1)
    w[:, :D, K : K + D] = ie[:btot, off : off + D * D].reshape(btot, D, D)  # W1
    w[:, D, K : K + D] = ie[:btot, off + D * D : off + D * D + D]  # b1

    chs = (
        comb.reshape(ncores, nbatch, g, 4, K, CC)
        .transpose(0, 1, 3, 4, 2, 5)       # c, bi, j, d, qq, col
        .astype(dt, copy=False)
    )
    return [
        {"ch": np.ascontiguousarray(chs[c]).reshape(nbatch, 4 * K, g * CC)}
        for c in range(ncores)
    ]


def unpack_output(results, nq=QUADS, g=G):
    """results: per-core {"yh": [nbatch, 4*D, g*N]} -> full (B, N, D) f32."""
    nbatch = nq // g
    yh = np.stack([r["yh"] for r in results])
    y = (
        yh.reshape(NCORES, nbatch, 4, D, g, N)
        .transpose(0, 1, 4, 2, 5, 3)            # c, bi, qq, j, n, e
    )
    return np.ascontiguousarray(y.reshape(NCORES * nq * 4, N, D))


_NC_CACHE = {}


def _get_nc(key=(QUADS, G)):
    if key not in _NC_CACHE:
        nq, g = key
        _NC_CACHE[key] = build_nc(nq=nq, g=g)
    return _NC_CACHE[key]


def kernel(user_emb, item_emb):
    nc = _get_nc()
    in_maps = pack_inputs(user_emb, item_emb)
    res = run_bass_kernel_spmd(nc, in_maps, core_ids=list(range(NCORES)))
    return unpack_output(res.results)


# revision 32
# speedup vs baseline: 1.8169x; 1.7771x over previous
"""CAN per-sample 2-layer MLP kernel for Trainium2 (8 NeuronCores, SPMD).

Computation (per sample b):
    x = user_emb[b]                           # (50, 16)
    W0, b0, W1, b1 unpacked from item_emb[b]  # (16,16),(16,),(16,16),(16,)
    y = relu(relu(x @ W0 + b0) @ W1 + b1)     # (50, 16)

Mapping:
  * Pure data parallel over 8 cores (2048 samples each).
  * Host packs x^T per sample with an appended ones-row (homogeneous
    coordinates); bias is folded into a 17x17 Wt0 = [[W0,0],[b0,1]] and a
    17x16 Wt1 = [[W1],[b1]] so `x_t @ Wt` applies bias, and the ones row
    self-propagates through layer 1 (relu(1)=1).
  * On chip: 4 samples share a 128-partition tile at 32-row strides so each
    sample's K=17 matmul runs at its own PE tile_position (32j, 32j); the
    four matmuls execute concurrently in distinct array quadrants.
  * x and W interleave in ONE combined DRAM tensor, batch-major so every
    DMA has a contiguous DRAM side. DMA batches are large (G quads) to
    amortize the ~2us per-lane completion latency; PSUM works in GS-quad
    sub-batches (one bank per tile). Input DMAs are issued 2 batches ahead
    on an explicitly chained SP queue so loads overlap compute.
  * Walrus codegen caps inline sync waits (DMACopy/Matmult: 1): a post-pass
    moves excess waits onto NoOps inserted before the instruction on the
    same queue. SBUF-side DMA APs must stay 2D ([row, nparts], [1, run]) -
    the DIRECT2D DMA struct cannot roll a free run across partitions.
"""

from contextlib import ExitStack

import numpy as np

import concourse.bass as bass
import concourse.mybir as mybir
from concourse import tile
from concourse.bass_utils import run_bass_kernel_spmd
from concourse.tile_rust import add_dep_helper

# Problem constants (hardcoded per contract)
B, N, D = 16384, 50, 16
NCORES = 8
BC = B // NCORES            # 2048 samples per core
K = D + 1                   # 17 rows: 16 features + homogeneous ones row
WC = K + D                  # 33 weight cols: 17 (layer0 incl ones col) + 16 (layer1)
CC = N + WC                 # 83 combined cols per quad-slot: x (50) then w (33)
QUADS = BC // 4             # 512 quads of 4 samples per core
G = 32                      # quads per DMA batch
GS = 8                      # quads per PSUM sub-batch (one bank)

F32 = mybir.dt.float32


def _strip_covered_waits(nc):
    """Remove, from DMACopy instructions, semaphore waits already guaranteed
    by an earlier instruction on the same engine queue. Coverage is killed
    for a sem from the point of any non-increment update (barrier resets)."""
    for fn in nc.m.functions:
        for blk in fn.blocks:
            seen = {}
            for ins in blk.instructions:
                si = ins.sync_info
                if si is None:
                    continue
                eng = ins.engine
                strippable = type(ins).__name__ == "InstDMACopy"
                kept = []
                changed = False
                for w in si.on_wait:
                    if (
                        strippable
                        and w.wait_mode == "sem-ge-imm"
                        and w.wait_reg is None
                        and seen.get((eng, w.id), -1) >= w.wait_value
                    ):
                        changed = True
                        continue
                    kept.append(w)
                for w in kept:
                    if w.wait_mode == "sem-ge-imm" and w.wait_reg is None:
                        key = (eng, w.id)
                        if seen.get(key, -1) < w.wait_value:
                            seen[key] = w.wait_value
                for u in si.on_update:
                    if u.update_mode != "sem-add-imm" or (
                        u.update_value is not None and u.update_value < 0
                    ):
                        for key in [k for k in seen if k[1] == u.id]:
                            del seen[key]
                if changed:
                    ins.sync_info = mybir.SyncInfo(
                        on_wait=kept, on_update=si.on_update
                    )


_WS_COUNT = [0]


def _split_excess_waits(nc, cap=1):
    """Move excess inline waits onto NoOps inserted immediately before, on
    the same engine queue - semantically identical (sequencers execute
    waits in order)."""
    for fn in nc.m.functions:
        for blk in fn.blocks:
            insts = blk.instructions
            i = 0
            while i < len(insts):
                ins = insts[i]
                si = ins.sync_info
                if si is None or len(si.on_wait) <= cap:
                    i += 1
                    continue
                waits = list(si.on_wait)
                keep, extra = waits[-cap:], waits[:-cap]
                ins.sync_info = mybir.SyncInfo(on_wait=keep, on_update=si.on_update)
                for w in extra:
                    _WS_COUNT[0] += 1
                    nop = mybir.InstNoOp(name=f"I-ws{_WS_COUNT[0]}", ins=[], outs=[])
                    nop.engine = ins.engine
                    nop.sync_info = mybir.SyncInfo(on_wait=[w], on_update=[])
                    insts.insert(i, nop)
                    i += 1
                i += 1


def build_nc(nq=QUADS, g=G, gs=GS, dt=F32, sim_mode=False):
    """Build the per-core Bass program.

    DRAM (per core), batch-major:
      ch [nbatch, 4*K, g*CC] : row K*j+d, col qq*CC+c = sample (bi*g+qq)*4+j
      yh [nbatch, 4*D, g*N]  : row D*j+e, col qq*N+n  = y[n, e] of same
    """
    assert nq % g == 0 and g % gs == 0
    nbatch = nq // g
    nsub = g // gs
    cf = CC * g              # ct data cols per batch
    xf = N * g               # yt data cols per batch
    sf = N * gs              # psum/ht cols per sub-batch
    cfp = cf + 8             # padded row widths: keep SBUF DMA APs 2D
    xfp = xf + 8

    nc = bass.Bass(
        "TRN2",
        target_bir_lowering=False,
        debug=False,
        detect_race_conditions=False,  # post-pass NoOps confuse its bookkeeping
    )
    ch = nc.dram_tensor("ch", [nbatch, 4 * K, cf], dt, kind="ExternalInput")
    yh = nc.dram_tensor("yh", [nbatch, 4 * D, xf], F32, kind="ExternalOutput")

    relu = mybir.ActivationFunctionType.Relu

    with tile.TileContext(nc) as tc, ExitStack() as ctx:
        cpool = ctx.enter_context(tc.tile_pool(name="cpool", bufs=3))
        hpool = ctx.enter_context(tc.tile_pool(name="hpool", bufs=3))
        ypool = ctx.enter_context(tc.tile_pool(name="ypool", bufs=2))
        pspool = ctx.enter_context(tc.tile_pool(name="ps", bufs=4, space="PSUM"))

        prev_sp = [None]

        def sp_chain(inst):
            # Pin SP issue order to emission order so prefetched loads are
            # dispatched before later batches' stores.
            if prev_sp[0] is not None:
                add_dep_helper(inst.ins, prev_sp[0].ins, sync=False,
                               reason="SP issue order")
            prev_sp[0] = inst
            return inst

        cts = {}

        def emit_in_dma(bi):
            ct = cpool.tile([128, cfp], dt, name="ct")
            cts[bi] = ct
            for j in range(4):
                # SWDGE (gpsimd): HWDGE assigns DRAM-sourced loads to a
                # single SDMA engine (observed: one engine 98% busy while
                # 15 idle); SWDGE sprays descriptors across engines by
                # destination partition.
                nc.gpsimd.dma_start(
                    bass.AP(ct.tensor, 32 * j * cfp, [[cfp, K], [1, cf]]),
                    bass.AP(ch, (bi * 4 * K + j * K) * cf, [[cf, K], [1, cf]]),
                )

        # 2-deep prefetch prologue
        emit_in_dma(0)
        if nbatch > 1:
            emit_in_dma(1)

        for bi in range(nbatch):
            if bi + 2 < nbatch:
                emit_in_dma(bi + 2)
            ct = cts.pop(bi)

            yt = ypool.tile([128, xfp], F32, name="yt")
            for s in range(nsub):
                ps1 = pspool.tile([128, sf], F32, name="ps1")
                if sim_mode:
                    nc.vector.memset(ps1[:, :], 0.0)
                for q in range(gs):
                    qq = s * gs + q
                    for j in range(4):
                        nc.tensor.matmul(
                            bass.AP(ps1.tensor, 32 * j * sf + q * N, [[sf, K], [1, N]]),
                            bass.AP(ct.tensor, 32 * j * cfp + qq * CC + N, [[cfp, K], [1, K]]),
                            bass.AP(ct.tensor, 32 * j * cfp + qq * CC, [[cfp, K], [1, N]]),
                            start=True,
                            stop=True,
                            tile_position=(32 * j, 32 * j),
                        )

                ht = hpool.tile([128, sf], dt, name="ht")
                nc.scalar.activation(ht[:, :], ps1[:, :], relu)

                ps2 = pspool.tile([128, sf], F32, name="ps2")
                if sim_mode:
                    nc.vector.memset(ps2[:, :], 0.0)
                for q in range(gs):
                    qq = s * gs + q
                    for j in range(4):
                        nc.tensor.matmul(
                            bass.AP(ps2.tensor, 32 * j * sf + q * N, [[sf, D], [1, N]]),
                            bass.AP(ct.tensor, 32 * j * cfp + qq * CC + N + K, [[cfp, K], [1, D]]),
                            bass.AP(ht.tensor, 32 * j * sf + q * N, [[sf, K], [1, N]]),
                            start=True,
                            stop=True,
                            tile_position=(32 * j, 32 * j),
                        )

                # relu2 writes this sub-batch's slice of yt (DVE)
                nc.vector.tensor_scalar_max(
                    bass.AP(yt.tensor, s * sf, [[xfp, 128], [1, sf]]),
                    ps2[:, :],
                    0.0,
                )

            for j in range(4):
                sp_chain(nc.sync.dma_start(
                    bass.AP(yh, (bi * 4 * D + j * D) * xf, [[xf, D], [1, xf]]),
                    bass.AP(yt.tensor, 32 * j * xfp, [[xfp, D], [1, xf]]),
                ))

    _strip_covered_waits(nc)
    _split_excess_waits(nc)
    return nc


def pack_inputs(user_emb, item_emb, nq=QUADS, g=G, dt=np.float32):
    """Shard + lay out inputs for the 8 cores. Returns list of in_maps."""
    ncores = NCORES
    nbatch = nq // g
    x = np.ascontiguousarray(user_emb, dtype=np.float32)
    ie = np.ascontiguousarray(item_emb, dtype=np.float32)
    btot = ncores * nq * 4

    comb = np.empty((btot, K, CC), dtype=np.float32)
    comb[:, :D, :N] = x[:btot].transpose(0, 2, 1)
    comb[:, D, :N] = 1.0
    w = comb[:, :, N:]
    w[:, :D, :D] = ie[:btot, : D * D].reshape(btot, D, D)          # W0
    w[:, D, :D] = ie[:btot, D * D : D * D + D]                     # b0
    w[:, :D, D] = 0.0
    w[:, D, D] = 1.0                                               # ones col
    off = D * (D + 1)
    w[:, :D, K : K + D] = ie[:btot, off : off + D * D].reshape(btot, D, D)  # W1
    w[:, D, K : K + D] = ie[:btot, off + D * D : off + D * D + D]  # b1

    chs = (
        comb.reshape(ncores, nbatch, g, 4, K, CC)
        .transpose(0, 1, 3, 4, 2, 5)       # c, bi, j, d, qq, col
        .astype(dt, copy=False)
    )
    return [
        {"ch": np.ascontiguousarray(chs[c]).reshape(nbatch, 4 * K, g * CC)}
        for c in range(ncores)
    ]


def unpack_output(results, nq=QUADS, g=G):
    """results: per-core {"yh": [nbatch, 4*D, g*N]} -> full (B, N, D) f32."""
    nbatch = nq // g
    yh = np.stack([r["yh"] for r in results])
    y = (
        yh.reshape(NCORES, nbatch, 4, D, g, N)
        .transpose(0, 1, 4, 2, 5, 3)            # c, bi, qq, j, n, e
    )
    return np.ascontiguousarray(y.reshape(NCORES * nq * 4, N, D))


_NC_CACHE = {}


def _get_nc(key=(QUADS, G)):
    if key not in _NC_CACHE:
        nq, g = key
        _NC_CACHE[key] = build_nc(nq=nq, g=g)
    return _NC_CACHE[key]


def kernel(user_emb, item_emb):
    nc = _get_nc()
    in_maps = pack_inputs(user_emb, item_emb)
    res = run_bass_kernel_spmd(nc, in_maps, core_ids=list(range(NCORES)))
    return unpack_output(res.results)


# revision 33
# speedup vs baseline: 1.9508x; 1.0737x over previous
"""CAN per-sample 2-layer MLP kernel for Trainium2 (8 NeuronCores, SPMD).

Computation (per sample b):
    x = user_emb[b]                           # (50, 16)
    W0, b0, W1, b1 unpacked from item_emb[b]  # (16,16),(16,),(16,16),(16,)
    y = relu(relu(x @ W0 + b0) @ W1 + b1)     # (50, 16)

Mapping:
  * Pure data parallel over 8 cores (2048 samples each).
  * Host packs x^T per sample with an appended ones-row (homogeneous
    coordinates); bias is folded into a 17x17 Wt0 = [[W0,0],[b0,1]] and a
    17x16 Wt1 = [[W1],[b1]] so `x_t @ Wt` applies bias, and the ones row
    self-propagates through layer 1 (relu(1)=1).
  * On chip: 4 samples share a 128-partition tile at 32-row strides so each
    sample's K=17 matmul runs at its own PE tile_position (32j, 32j); the
    four matmuls execute concurrently in distinct array quadrants.
  * x and W interleave in ONE combined DRAM tensor, batch-major so every
    DMA has a contiguous DRAM side. DMA batches are large (G quads) to
    amortize the ~2us per-lane completion latency; PSUM works in GS-quad
    sub-batches (one bank per tile). Input DMAs are issued 2 batches ahead
    on an explicitly chained SP queue so loads overlap compute.
  * Walrus codegen caps inline sync waits (DMACopy/Matmult: 1): a post-pass
    moves excess waits onto NoOps inserted before the instruction on the
    same queue. SBUF-side DMA APs must stay 2D ([row, nparts], [1, run]) -
    the DIRECT2D DMA struct cannot roll a free run across partitions.
"""

from contextlib import ExitStack

import numpy as np

import concourse.bass as bass
import concourse.mybir as mybir
from concourse import tile
from concourse.bass_utils import run_bass_kernel_spmd
from concourse.tile_rust import add_dep_helper

# Problem constants (hardcoded per contract)
B, N, D = 16384, 50, 16
NCORES = 8
BC = B // NCORES            # 2048 samples per core
K = D + 1                   # 17 rows: 16 features + homogeneous ones row
WC = K + D                  # 33 weight cols: 17 (layer0 incl ones col) + 16 (layer1)
CC = N + WC                 # 83 combined cols per quad-slot: x (50) then w (33)
QUADS = BC // 4             # 512 quads of 4 samples per core
G = 32                      # quads per DMA batch
GS = 8                      # quads per PSUM sub-batch (one bank)

F32 = mybir.dt.float32


def _strip_covered_waits(nc):
    """Remove, from DMACopy instructions, semaphore waits already guaranteed
    by an earlier instruction on the same engine queue. Coverage is killed
    for a sem from the point of any non-increment update (barrier resets)."""
    for fn in nc.m.functions:
        for blk in fn.blocks:
            seen = {}
            for ins in blk.instructions:
                si = ins.sync_info
                if si is None:
                    continue
                eng = ins.engine
                strippable = type(ins).__name__ == "InstDMACopy"
                kept = []
                changed = False
                for w in si.on_wait:
                    if (
                        strippable
                        and w.wait_mode == "sem-ge-imm"
                        and w.wait_reg is None
                        and seen.get((eng, w.id), -1) >= w.wait_value
                    ):
                        changed = True
                        continue
                    kept.append(w)
                for w in kept:
                    if w.wait_mode == "sem-ge-imm" and w.wait_reg is None:
                        key = (eng, w.id)
                        if seen.get(key, -1) < w.wait_value:
                            seen[key] = w.wait_value
                for u in si.on_update:
                    if u.update_mode != "sem-add-imm" or (
                        u.update_value is not None and u.update_value < 0
                    ):
                        for key in [k for k in seen if k[1] == u.id]:
                            del seen[key]
                if changed:
                    ins.sync_info = mybir.SyncInfo(
                        on_wait=kept, on_update=si.on_update
                    )


_WS_COUNT = [0]


def _split_excess_waits(nc, cap=1):
    """Move excess inline waits onto NoOps inserted immediately before, on
    the same engine queue - semantically identical (sequencers execute
    waits in order)."""
    for fn in nc.m.functions:
        for blk in fn.blocks:
            insts = blk.instructions
            i = 0
            while i < len(insts):
                ins = insts[i]
                si = ins.sync_info
                if si is None or len(si.on_wait) <= cap:
                    i += 1
                    continue
                waits = list(si.on_wait)
                keep, extra = waits[-cap:], waits[:-cap]
                ins.sync_info = mybir.SyncInfo(on_wait=keep, on_update=si.on_update)
                for w in extra:
                    _WS_COUNT[0] += 1
                    nop = mybir.InstNoOp(name=f"I-ws{_WS_COUNT[0]}", ins=[], outs=[])
                    nop.engine = ins.engine
                    nop.sync_info = mybir.SyncInfo(on_wait=[w], on_update=[])
                    insts.insert(i, nop)
                    i += 1
                i += 1


def build_nc(nq=QUADS, g=G, gs=GS, dt=F32, sim_mode=False):
    """Build the per-core Bass program.

    DRAM (per core), batch-major:
      ch [nbatch, 4*K, g*CC] : row K*j+d, col qq*CC+c = sample (bi*g+qq)*4+j
      yh [nbatch, 4*D, g*N]  : row D*j+e, col qq*N+n  = y[n, e] of same
    """
    assert nq % g == 0 and g % gs == 0
    nbatch = nq // g
    nsub = g // gs
    cf = CC * g              # ct data cols per batch
    xf = N * g               # yt data cols per batch
    sf = N * gs              # psum/ht cols per sub-batch
    cfp = cf + 8             # padded row widths: keep SBUF DMA APs 2D
    xfp = xf + 8

    nc = bass.Bass(
        "TRN2",
        target_bir_lowering=False,
        debug=False,
        detect_race_conditions=False,  # post-pass NoOps confuse its bookkeeping
    )
    ch = nc.dram_tensor("ch", [nbatch, 4 * K, cf], dt, kind="ExternalInput")
    yh = nc.dram_tensor("yh", [nbatch, 4 * D, xf], F32, kind="ExternalOutput")

    relu = mybir.ActivationFunctionType.Relu

    with tile.TileContext(nc) as tc, ExitStack() as ctx:
        cpool = ctx.enter_context(tc.tile_pool(name="cpool", bufs=3))
        hpool = ctx.enter_context(tc.tile_pool(name="hpool", bufs=3))
        ypool = ctx.enter_context(tc.tile_pool(name="ypool", bufs=2))
        pspool = ctx.enter_context(tc.tile_pool(name="ps", bufs=4, space="PSUM"))

        prev_sp = [None]

        def sp_chain(inst):
            # Pin SP issue order to emission order so prefetched loads are
            # dispatched before later batches' stores.
            if prev_sp[0] is not None:
                add_dep_helper(inst.ins, prev_sp[0].ins, sync=False,
                               reason="SP issue order")
            prev_sp[0] = inst
            return inst

        cts = {}

        def emit_in_dma(bi):
            ct = cpool.tile([128, cfp], dt, name="ct")
            cts[bi] = ct
            for j in range(4):
                # SWDGE (gpsimd): HWDGE assigns DRAM-sourced loads to a
                # single SDMA engine (observed: one engine 98% busy while
                # 15 idle); SWDGE sprays descriptors across engines by
                # destination partition.
                nc.gpsimd.dma_start(
                    bass.AP(ct.tensor, 32 * j * cfp, [[cfp, K], [1, cf]]),
                    bass.AP(ch, (bi * 4 * K + j * K) * cf, [[cf, K], [1, cf]]),
                )

        # 2-deep prefetch prologue
        emit_in_dma(0)
        if nbatch > 1:
            emit_in_dma(1)

        # Software-pipelined emission with one-sub-batch skew: the PE queue
        # is in-order, so L2(s) (which waits on relu1(s)) is emitted AFTER
        # L1(s+1) - the PE computes layer 1 of the next sub-batch while
        # relu1(s) runs on ACT, instead of stalling.
        subs = [(bi, s) for bi in range(nbatch) for s in range(nsub)]
        state = {}   # ss -> (ct, yt, ps1, ht)
        yts = {}

        def emit_l1(ss):
            bi, s = subs[ss]
            if s == 0:
                if bi + 2 < nbatch:
                    emit_in_dma(bi + 2)
                yts[bi] = ypool.tile([128, xfp], F32, name="yt")
            ct = cts[bi]
            ps1 = pspool.tile([128, sf], F32, name="ps1")
            if sim_mode:
                nc.vector.memset(ps1[:, :], 0.0)
            for q in range(gs):
                qq = s * gs + q
                for j in range(4):
                    nc.tensor.matmul(
                        bass.AP(ps1.tensor, 32 * j * sf + q * N, [[sf, K], [1, N]]),
                        bass.AP(ct.tensor, 32 * j * cfp + qq * CC + N, [[cfp, K], [1, K]]),
                        bass.AP(ct.tensor, 32 * j * cfp + qq * CC, [[cfp, K], [1, N]]),
                        start=True,
                        stop=True,
                        tile_position=(32 * j, 32 * j),
                    )
            ht = hpool.tile([128, sf], dt, name="ht")
            nc.scalar.activation(ht[:, :], ps1[:, :], relu)
            state[ss] = (ct, ht)

        def emit_l2(ss):
            bi, s = subs[ss]
            ct, ht = state.pop(ss)
            yt = yts[bi]
            ps2 = pspool.tile([128, sf], F32, name="ps2")
            if sim_mode:
                nc.vector.memset(ps2[:, :], 0.0)
            for q in range(gs):
                qq = s * gs + q
                for j in range(4):
                    nc.tensor.matmul(
                        bass.AP(ps2.tensor, 32 * j * sf + q * N, [[sf, D], [1, N]]),
                        bass.AP(ct.tensor, 32 * j * cfp + qq * CC + N + K, [[cfp, K], [1, D]]),
                        bass.AP(ht.tensor, 32 * j * sf + q * N, [[sf, K], [1, N]]),
                        start=True,
                        stop=True,
                        tile_position=(32 * j, 32 * j),
                    )
            # relu2 writes this sub-batch's slice of yt (DVE)
            nc.vector.tensor_scalar_max(
                bass.AP(yt.tensor, s * sf, [[xfp, 128], [1, sf]]),
                ps2[:, :],
                0.0,
            )
            if s == nsub - 1:
                for j in range(4):
                    sp_chain(nc.sync.dma_start(
                        bass.AP(yh, (bi * 4 * D + j * D) * xf, [[xf, D], [1, xf]]),
                        bass.AP(yt.tensor, 32 * j * xfp, [[xfp, D], [1, xf]]),
                    ))
                cts.pop(bi)

        for idx in range(len(subs) + 1):
            if idx < len(subs):
                emit_l1(idx)
            if idx >= 1:
                emit_l2(idx - 1)

    _strip_covered_waits(nc)
    _split_excess_waits(nc)
    return nc


def pack_inputs(user_emb, item_emb, nq=QUADS, g=G, dt=np.float32):
    """Shard + lay out inputs for the 8 cores. Returns list of in_maps."""
    ncores = NCORES
    nbatch = nq // g
    x = np.ascontiguousarray(user_emb, dtype=np.float32)
    ie = np.ascontiguousarray(item_emb, dtype=np.float32)
    btot = ncores * nq * 4

    comb = np.empty((btot, K, CC), dtype=np.float32)
    comb[:, :D, :N] = x[:btot].transpose(0, 2, 1)
    comb[:, D, :N] = 1.0
    w = comb[:, :, N:]
    w[:, :D, :D] = ie[:btot, : D * D].reshape(btot, D, D)          # W0
    w[:, D, :D] = ie[:btot, D * D : D * D + D]                     # b0
    w[:, :D, D] = 0.0
    w[:, D, D] = 1.0                                               # ones col
    off = D * (D + 1)
    w[:, :D, K : K + D] = ie[:btot, off : off + D * D].reshape(btot, D, D)  # W1
    w[:, D, K : K + D] = ie[:btot, off + D * D : off + D * D + D]  # b1

    chs = (
        comb.reshape(ncores, nbatch, g, 4, K, CC)
        .transpose(0, 1, 3, 4, 2, 5)       # c, bi, j, d, qq, col
        .astype(dt, copy=False)
    )
    return [
        {"ch": np.ascontiguousarray(chs[c]).reshape(nbatch, 4 * K, g * CC)}
        for c in range(ncores)
    ]


def unpack_output(results, nq=QUADS, g=G):
    """results: per-core {"yh": [nbatch, 4*D, g*N]} -> full (B, N, D) f32."""
    nbatch = nq // g
    yh = np.stack([r["yh"] for r in results])
    y = (
        yh.reshape(NCORES, nbatch, 4, D, g, N)
        .transpose(0, 1, 4, 2, 5, 3)            # c, bi, qq, j, n, e
    )
    return np.ascontiguousarray(y.reshape(NCORES * nq * 4, N, D))


_NC_CACHE = {}


def _get_nc(key=(QUADS, G)):
    if key not in _NC_CACHE:
        nq, g = key
        _NC_CACHE[key] = build_nc(nq=nq, g=g)
    return _NC_CACHE[key]


def kernel(user_emb, item_emb):
    nc = _get_nc()
    in_maps = pack_inputs(user_emb, item_emb)
    res = run_bass_kernel_spmd(nc, in_maps, core_ids=list(range(NCORES)))
    return unpack_output(res.results)
